# revision 36
# baseline (speedup 1.0000x reference)
"""Trainium2 Bass kernel for fused MHA block (nn_MultiHeadAttention_7636451852747).

Reference math (B=2, S=4096, D=512, H=8, hd=64):
    q = (x @ Wq + bq).view(B, H, 4096, 64)   # torch-style view, no transpose
    scores = q @ k^T / 8; attn = softmax(scores) @ v -> reshape(B, S, D)
    y = LayerNorm(x + attn) * gamma + beta

Structure: the .view means head h of batch b reads only rows [512h, 512h+512)
of x[b]; the problem splits into 16 independent [512,512] chunks, 2 per core.

Design, driven by the TRN2 timeline cost model:
  * Softmax exp of 16.8M scores/chunk is the bottleneck (only ACT and DVE
    can read PSUM; GPSIMD cannot).  Exp is split across ACT (true exp ->
    fp8e5m2 with bias = ln(scale)) and DVE (Schraudolph bit-trick:
    round(score*EXP_A + EXP_B) as int8 IS the e5m2 weight, scale-matched
    to the ACT path so weights mix within a softmax row).  A greedy
    cost-balancer assigns each exp tile to the less-loaded engine.
  * q/k path runs in bf16 (projection, storage, scores) — fp8 q/k noise
    multiplied into dominant softmax weights and nearly doubled the absmax
    error.  Scores use the 2x row-packed [64,128]x[64,512] layout with a
    partition-swapped qTs copy (tile_position pairing).
  * The attention matmul is fp8 DoubleRow (0.5 cycles/row, 2 k-tiles per
    pass) with E^T e5m2 as the stationary operand and v e4m3 (plus a
    column of ones for the softmax denominator) as the moving operand, so
    the output lands in natural [s, dv] layout: no PE transposes, no
    psum->sbuf attn copy.  E tiles for one jq-strip live in SBUF (strict
    per-strip barrier), accumulators are [128,65] psum tiles finalized by
    one reciprocal + one scalar_tensor_tensor (divide + residual add).
  * LayerNorm: DVE bn_stats per 128-row block (interleaved with finalize),
    rstd via ACT Sqrt + DVE reciprocal, y = (h*rstd+bco) on ACT (Identity
    with per-partition scale/bias), *gamma on DVE, +beta alternating
    DVE/GPSIMD to shorten the endgame tail.
  * PSUM: 6 banks triple-buffer the score pairs, 1+1 banks double-buffer
    the projection pipeline (alternating two pools), the tail attention
    borrows the idle score pool for 3-deep pipelining.
"""
import numpy as np
import ml_dtypes
from contextlib import ExitStack

_STATE = {}


def _imports():
    global bass, bacc, tile, mybir, bass_utils, F32, BF16, I8, E4, E5, ALU, ACTF, DR
    import concourse.bass as bass
    import concourse.bacc as bacc
    import concourse.tile as tile
    from concourse import mybir
    from concourse import bass_utils
    F32 = mybir.dt.float32
    BF16 = mybir.dt.bfloat16
    I8 = mybir.dt.int8
    E4 = mybir.dt.float8e4
    E5 = mybir.dt.float8e5
    ALU = mybir.AluOpType
    ACTF = mybir.ActivationFunctionType
    DR = mybir.MatmulPerfMode.DoubleRow


N_CORES = 8
CH = 2           # chunks per core
S = 512          # rows per chunk
D = 512          # model dim
EPS = 1e-5

# Schraudolph-e5m2 constants (calibrated offline vs true softmax):
#   i8 = round(score * EXP_A + EXP_B); bits are the e5m2 weight
#   ACT path: exp(score/8 + LN_S) in e5m2 matches the Schraudolph scale.
EXP_A = 4 * np.log2(np.e) / 8        # 0.7213475
EXP_B = 58.0
LN_S = -0.3095

# emit-time engine cost estimates (us) for the greedy ACT/DVE balancer
C_ACT_EXP = 1.098
C_DVE_EXP = 1.262
C_ACT_CONV = 0.672
C_DVE_CONV = 0.730
C_DVE_VCONV = 0.80
C_DVE_FIN = 0.40
C_DVE_LN = 5.2


def _emit(nc, tc, ctx):
    xtb_d = nc.dram_tensor("xtb", [CH, 128, 2048], BF16, kind="ExternalInput").ap()
    xf_d = nc.dram_tensor("xf", [CH, 128, 2048], F32, kind="ExternalInput").ap()
    wb_d = {n: nc.dram_tensor(n, [128, 2048], BF16, kind="ExternalInput").ap()
            for n in ("wbq", "wbk", "wbv")}
    bqt_d = nc.dram_tensor("bqt", [128, 4], F32, kind="ExternalInput").ap()
    bkt_d = nc.dram_tensor("bkt", [128, 4], F32, kind="ExternalInput").ap()
    bvb_d = nc.dram_tensor("bvb", [128, D], F32, kind="ExternalInput").ap()
    gb_d = nc.dram_tensor("gb", [128, D], F32, kind="ExternalInput").ap()
    bb_d = nc.dram_tensor("bb", [128, D], F32, kind="ExternalInput").ap()
    y_d = nc.dram_tensor("y", [CH, S, D], F32, kind="ExternalOutput").ap()

    consts = ctx.enter_context(tc.tile_pool(name="consts", bufs=1))
    chunkp = ctx.enter_context(tc.tile_pool(name="chunk", bufs=1))
    epool = ctx.enter_context(tc.tile_pool(name="epool", bufs=3))
    ypool = ctx.enter_context(tc.tile_pool(name="ypool", bufs=4))
    small = ctx.enter_context(tc.tile_pool(name="small", bufs=4))
    ps_proj = ctx.enter_context(tc.tile_pool(name="ps_proj", bufs=1, space="PSUM"))
    ps_score = ctx.enter_context(tc.tile_pool(name="ps_score", bufs=3, space="PSUM"))
    ps_attn = ctx.enter_context(tc.tile_pool(name="ps_attn", bufs=1, space="PSUM"))

    wb = {n: consts.tile([128, 2048], BF16, tag=n, name=f"w_{n}")
          for n in ("wbq", "wbk", "wbv")}
    bqt = consts.tile([128, 4], F32, tag="bqt")
    bkt = consts.tile([128, 4], F32, tag="bkt")
    bvb = consts.tile([128, D], F32, tag="bvb")
    gb = consts.tile([128, D], F32, tag="gb")
    bb = consts.tile([128, D], F32, tag="bb")
    lns = consts.tile([128, 1], F32, tag="lns")

    def consts_dma():
        nc.sync.dma_start(wb["wbk"][:], wb_d["wbk"][:])
        nc.vector.memset(lns[:], LN_S)
        # warm the ACT function table while DMAs stream (1.3us one-time load)
        warm = consts.tile([128, 1], F32, tag="warm")
        nc.scalar.activation(warm[:], lns[:], ACTF.Exp)

    def consts_dma_late():
        nc.sync.dma_start(bkt[:], bkt_d[:])
        nc.sync.dma_start(bqt[:], bqt_d[:])
        nc.sync.dma_start(wb["wbq"][:], wb_d["wbq"][:])
        nc.sync.dma_start(wb["wbv"][:], wb_d["wbv"][:])
        nc.sync.dma_start(bvb[:], bvb_d[:])
        nc.sync.dma_start(gb[:], gb_d[:])
        nc.sync.dma_start(bb[:], bb_d[:])

    # greedy engine balancer (estimated cumulative us per engine)
    bal = {"act": 0.0, "dve": 0.0}

    def pick_engine():
        return "act" if bal["act"] <= bal["dve"] else "dve"

    st = [{} for _ in range(CH)]

    def loads(c):
        s = st[c]
        s["xtb"] = chunkp.tile([128, 2048], BF16, tag=f"xtb_{c}", name=f"xtb{c}")
        s["xf"] = chunkp.tile([128, 2048], F32, tag=f"xf_{c}", name=f"xf{c}")
        nc.sync.dma_start(s["xtb"][:], xtb_d[c])
        s["qT"] = chunkp.tile([128, 2048], BF16, tag=f"qT_{c}", name=f"qT{c}")
        s["qTs"] = chunkp.tile([128, 2048], BF16, tag=f"qTs_{c}", name=f"qTs{c}")
        s["kT"] = chunkp.tile([128, 2048], BF16, tag=f"kT_{c}", name=f"kT{c}")
        s["vp"] = chunkp.tile([128, 4 * 520], E4, tag=f"vp_{c}", name=f"vp{c}")
        s["h"] = chunkp.tile([128, 2048], F32, tag=f"h_{c}", name=f"h{c}")

    def proj(c):
        s = st[c]
        xt = s["xtb"]
        n = 0
        for which in ("k", "q", "v"):
            for t in range(4):
                # alternate between the two single-buffer psum pools so the
                # PE->convert chain is double-buffered
                pool_, tag_ = ((ps_proj, "proj") if n % 2 == 0 else
                               (ps_attn, "pa"))
                n += 1
                pp = pool_.tile([128, D], F32, tag=tag_,
                                name=f"pp{c}_{which}{t}")
                if which == "v":
                    for mt in range(4):
                        nc.tensor.matmul(
                            pp[:],
                            xt[:, 512 * mt + 128 * t:512 * mt + 128 * t + 128],
                            wb["wbv"][:, 512 * mt:512 * (mt + 1)],
                            start=(mt == 0), stop=(mt == 3))
                    blk = s["vp"][:].rearrange("p (t j c) -> p t j c", j=8, c=65)
                    nc.vector.tensor_tensor(
                        blk[:, t, :, 0:64],
                        pp[:].rearrange("p (j c) -> p j c", c=64),
                        bvb[:].rearrange("p (j c) -> p j c", c=64), op=ALU.add)
                    nc.vector.memset(blk[:, t, :, 64], 1.0)
                else:
                    wname = "wbq" if which == "q" else "wbk"
                    dst = s["qT"] if which == "q" else s["kT"]
                    bias = bqt if which == "q" else bkt
                    for mt in range(4):
                        nc.tensor.matmul(
                            pp[:],
                            wb[wname][:, 512 * mt + 128 * t:512 * mt + 128 * t + 128],
                            xt[:, 512 * mt:512 * (mt + 1)],
                            start=(mt == 0), stop=(mt == 3))
                    eng = pick_engine()
                    if eng == "act":
                        nc.scalar.activation(dst[:, 512 * t:512 * (t + 1)],
                                             pp[:], ACTF.Identity,
                                             bias=bias[:, t:t + 1])
                        bal["act"] += C_ACT_CONV
                    else:
                        nc.vector.tensor_scalar(dst[:, 512 * t:512 * (t + 1)],
                                                pp[:], bias[:, t:t + 1], None,
                                                op0=ALU.add)
                        bal["dve"] += C_DVE_CONV
                    if which == "q":
                        # partition-swapped copy so score matmul rhs can sit at
                        # either PE tile row base (baseline qTs trick)
                        nc.sync.dma_start(
                            s["qTs"][64:128, 512 * t:512 * (t + 1)],
                            dst[0:64, 512 * t:512 * (t + 1)])
                        nc.sync.dma_start(
                            s["qTs"][0:64, 512 * t:512 * (t + 1)],
                            dst[64:128, 512 * t:512 * (t + 1)])

    # net fixed non-exp work difference (DVE minus ACT), spread as a
    # per-exp-tile handicap so the greedy split tilts smoothly
    # ~fixed_dve/(CH*128) from the cost estimates; the exact value was
    # tuned against TimelineSim (the schedule landscape is noisy at ~1us)
    handicap = 0.15

    def emit_exp(dst, ps):
        bal["dve"] += handicap
        eng = pick_engine()
        if eng == "act":
            nc.scalar.activation(dst, ps, ACTF.Exp, scale=0.125, bias=lns[:])
            bal["act"] += C_ACT_EXP
        else:
            nc.vector.tensor_scalar(dst.bitcast(I8), ps, EXP_A, EXP_B,
                                    op0=ALU.mult, op1=ALU.add)
            bal["dve"] += C_DVE_EXP

    def strips(c, jp):
        """Scores + exp for jq pair (2jp, 2jp+1): 16 (r,jku) psum pairs.

        bf16 scores, row-packed: jk-even rows via kT[0:64] at PE tile (0,0),
        jk-odd via kT[64:128] at (64,0); qT/qTs supply the rhs at the
        matching partition base (baseline pattern)."""
        s = st[c]
        qT, qTs, kT = s["qT"], s["qTs"], s["kT"]

        def qrhs(jq, par):
            src = qT if (jq % 2) == par else qTs
            return src[64 * par:64 * par + 64,
                       512 * (jq // 2):512 * (jq // 2) + 512]

        ets = {}
        for r in range(4):
            for jku in range(4):
                koff = 512 * jku + 128 * r
                ps0 = ps_score.tile([128, 1024], F32, tag="sps",
                                    name=f"s0_{c}_{jp}_{r}_{jku}")
                ps1 = ps_score.tile([128, 1024], F32, tag="sps",
                                    name=f"s1_{c}_{jp}_{r}_{jku}")
                for pjq in range(2):
                    jq = 2 * jp + pjq
                    nc.tensor.matmul(ps0[:, 512 * pjq:512 * (pjq + 1)],
                                     kT[0:64, koff:koff + 128], qrhs(jq, 0),
                                     start=True, stop=True,
                                     tile_position=(0, 0))
                for pjq in range(2):
                    jq = 2 * jp + pjq
                    nc.tensor.matmul(ps1[:, 512 * pjq:512 * (pjq + 1)],
                                     kT[64:128, koff:koff + 128], qrhs(jq, 1),
                                     start=True, stop=True,
                                     tile_position=(64, 0))
                et = epool.tile([128, 2048], E5, tag=f"e{4 * r + jku}",
                                name=f"e_{c}_{jp}_{r}_{jku}")
                emit_exp(et[:, 0:1024], ps0[:])
                emit_exp(et[:, 1024:2048], ps1[:])
                ets[(r, jku)] = et
        return ets

    def attn_fin(c, jp, ets, use_sps=False):
        """Flipped attention (E^T stationary) + finalize into h.

        use_sps: draw the accumulators from the (then idle) score psum pool
        for 3-deep pipelining — only safe when no more scores will run."""
        s = st[c]
        vp_v = s["vp"][:].rearrange("p (t j c) -> p t j c", j=8, c=65)
        for sb in range(4):
            for pjq in range(2):
                jq = 2 * jp + pjq
                if use_sps:
                    pa = ps_score.tile([128, 1024], F32, tag="sps",
                                       name=f"pa_{c}_{jp}_{sb}_{pjq}")
                else:
                    pa = ps_attn.tile([128, 512], F32, tag="pa",
                                      name=f"pa_{c}_{jp}_{sb}_{pjq}")
                k = 0
                for r in range(4):
                    for jku in range(4):
                        et = ets[(r, jku)]
                        lhsT = et[:].rearrange("p (i m) -> p i m", i=2)[
                            :, :, 512 * pjq + 128 * sb:512 * pjq + 128 * sb + 128]
                        rhs = vp_v[:, r, 2 * jku:2 * jku + 2, :]
                        nc.tensor.matmul(pa[:, 0:65], lhsT, rhs,
                                         start=(k == 0), stop=(k == 15),
                                         skip_group_check=True, perf_mode=DR)
                        k += 1
                rcp = small.tile([128, 1], F32, tag="rcp",
                                 name=f"rcp_{c}_{jp}_{sb}_{pjq}")
                nc.vector.reciprocal(rcp[:], pa[:, 64:65])
                off = 512 * sb + 64 * jq
                nc.vector.scalar_tensor_tensor(
                    s["h"][:, off:off + 64], pa[:, 0:64], rcp[:],
                    s["xf"][:, off:off + 64], op0=ALU.mult, op1=ALU.add)

    def layer_norm(c):
        """LayerNorm on DVE; Newton rsqrt batched across the 4 s-blocks."""
        s = st[c]
        h = s["h"]
        mvall = small.tile([128, 8], F32, tag="mvall", name=f"mv{c}")
        for b in range(4):
            st6 = small.tile([128, 6], F32, tag="st6", name=f"st6_{c}_{b}")
            nc.vector.bn_stats(st6[:], h[:, 512 * b:512 * (b + 1)])
            nc.vector.bn_aggr(mvall[:, 2 * b:2 * b + 2], st6[:])
        mean4 = mvall[:].rearrange("p (b two) -> p b two", two=2)[:, :, 0]
        var4 = mvall[:].rearrange("p (b two) -> p b two", two=2)[:, :, 1]
        t4 = small.tile([128, 4], F32, tag="t4", name=f"t4_{c}")
        nc.vector.tensor_scalar_add(t4[:], var4, EPS)
        sq4 = small.tile([128, 4], F32, tag="sq4", name=f"sq4_{c}")
        nc.scalar.activation(sq4[:], t4[:], ACTF.Sqrt)
        rstd = small.tile([128, 4], F32, tag="rstd", name=f"rstd{c}")
        nc.vector.reciprocal(rstd[:], sq4[:])
        bco = small.tile([128, 4], F32, tag="bco", name=f"bco{c}")
        nc.vector.tensor_tensor(bco[:], mean4, rstd[:], op=ALU.mult)
        nc.vector.tensor_scalar_mul(bco[:], bco[:], -1.0)
        for b in range(4):
            yt = ypool.tile([128, D], F32, tag="yt", name=f"yt{c}_{b}")
            # stage 1 (h*rstd + bco) on ACT to shorten the DVE-only tail
            nc.scalar.activation(yt[:], h[:, 512 * b:512 * (b + 1)],
                                 ACTF.Identity, bias=bco[:, b:b + 1],
                                 scale=rstd[:, b:b + 1])
            nc.vector.tensor_tensor(yt[:], yt[:], gb[:], op=ALU.mult)
            eng_bb = nc.gpsimd if b % 2 == 0 else nc.vector
            eng_bb.tensor_tensor(yt[:], yt[:], bb[:], op=ALU.add)
            nc.sync.dma_start(y_d[c, 128 * b:128 * (b + 1), :], yt[:])

    # ---- emission schedule: PE order = proj(0), scores(0,0..3) with
    # attn(jp-1) slotted between strips, proj(1), attn(0,3), scores(1,*) ...
    # The final strips gets an extra DVE handicap so ACT absorbs more of the
    # last exp wave while DVE runs finalize+LN with nothing after it.
    consts_dma()
    loads(0)
    consts_dma_late()
    proj(0)
    loads(1)
    nc.sync.dma_start(st[0]["xf"][:], xf_d[0])
    nc.sync.dma_start(st[1]["xf"][:], xf_d[1])
    pend = None   # (c, jp, ets) awaiting attn+finalize
    for c in range(CH):
        if c == 1:
            proj(1)
            attn_fin(*pend)
            pend = None
            layer_norm(0)
        for jp in range(4):
            ets = strips(c, jp)
            if pend is not None:
                attn_fin(*pend)
            pend = (c, jp, ets)
    attn_fin(*pend, use_sps=True)
    layer_norm(1)


def build():
    if "nc" in _STATE:
        return _STATE["nc"]
    _imports()
    nc = bacc.Bacc("TRN2", target_bir_lowering=False, debug=False,
                   num_devices=N_CORES)
    with tile.TileContext(nc) as tc:
        with ExitStack() as ctx:
            _emit(nc, tc, ctx)
    nc.compile()
    _STATE["nc"] = nc
    return nc


def host_inputs(Wq, bq, Wk, bk, Wv, bv, gamma, beta):
    """Shared per-core constant inputs (everything except x chunks)."""
    bf = ml_dtypes.bfloat16

    def pack_w(W):
        # [p, (mt, m)]: row 128*mt + p of W at free offset 512*mt + m
        W = np.asarray(W, np.float32)
        return np.ascontiguousarray(
            W.reshape(4, 128, 512).transpose(1, 0, 2).reshape(128, 2048)
        ).astype(bf)

    def bias_t(b):
        return np.ascontiguousarray(
            np.asarray(b, np.float32).reshape(4, 128).T)

    return {
        "wbq": pack_w(Wq), "wbk": pack_w(Wk), "wbv": pack_w(Wv),
        "bqt": bias_t(bq), "bkt": bias_t(bk),
        "bvb": np.broadcast_to(np.asarray(bv, np.float32), (128, D)).copy(),
        "gb": np.broadcast_to(np.asarray(gamma, np.float32), (128, D)).copy(),
        "bb": np.broadcast_to(np.asarray(beta, np.float32), (128, D)).copy(),
    }


def kernel(x, Wq, bq, Wk, bk, Wv, bv, gamma, beta):
    _imports()
    nc = build()
    bf = ml_dtypes.bfloat16
    x = np.asarray(x, np.float32)
    B, Sfull, Dm = x.shape
    chunks = x.reshape(B * 8, S, D)  # chunk c = (b = c//8, head = c%8)
    base = host_inputs(Wq=Wq, bq=bq, Wk=Wk, bk=bk, Wv=Wv, bv=bv,
                       gamma=gamma, beta=beta)
    in_maps = []
    for i in range(N_CORES):
        xc = chunks[2 * i:2 * i + 2]                       # [2, 512, 512]
        xtb = np.ascontiguousarray(
            xc.transpose(0, 2, 1).reshape(CH, 4, 128, S).transpose(0, 2, 1, 3)
            .reshape(CH, 128, 2048)).astype(bf)            # x^T, m-tile-major
        xf = np.ascontiguousarray(
            xc.reshape(CH, 4, 128, D).transpose(0, 2, 1, 3)
            .reshape(CH, 128, 2048))                       # residual, s-block-major
        m = dict(base)
        m["xtb"] = xtb
        m["xf"] = xf
        in_maps.append(m)
    res = bass_utils.run_bass_kernel_spmd(nc, in_maps, core_ids=list(range(N_CORES)))
    out_chunks = np.empty((B * 8, S, D), np.float32)
    for i in range(N_CORES):
        out_chunks[2 * i:2 * i + 2] = res.results[i]["y"]
    return out_chunks.reshape(B, Sfull, Dm)


# revision 40
# speedup vs baseline: 1.0126x; 1.0126x over previous
"""Trainium2 Bass kernel for fused MHA block (nn_MultiHeadAttention_7636451852747).

Reference math (B=2, S=4096, D=512, H=8, hd=64):
    q = (x @ Wq + bq).view(B, H, 4096, 64)   # torch-style view, no transpose
    scores = q @ k^T / 8; attn = softmax(scores) @ v -> reshape(B, S, D)
    y = LayerNorm(x + attn) * gamma + beta

Structure: the .view means head h of batch b reads only rows [512h, 512h+512)
of x[b]; the problem splits into 16 independent [512,512] chunks, 2 per core.

Design, driven by the TRN2 timeline cost model:
  * Softmax exp of 16.8M scores/chunk is the bottleneck (only ACT and DVE
    can read PSUM; GPSIMD cannot).  Exp is split across ACT (true exp ->
    fp8e5m2 with bias = ln(scale)) and DVE (Schraudolph bit-trick:
    round(score*EXP_A + EXP_B) as int8 IS the e5m2 weight, scale-matched
    to the ACT path so weights mix within a softmax row).  A greedy
    cost-balancer assigns each exp tile to the less-loaded engine.
  * q/k path runs in bf16 (projection, storage, scores) — fp8 q/k noise
    multiplied into dominant softmax weights and nearly doubled the absmax
    error.  Scores use the 2x row-packed [64,128]x[64,512] layout with a
    partition-swapped qTs copy (tile_position pairing).
  * The attention matmul is fp8 DoubleRow (0.5 cycles/row, 2 k-tiles per
    pass) with E^T e5m2 as the stationary operand and v e4m3 (plus a
    column of ones for the softmax denominator) as the moving operand, so
    the output lands in natural [s, dv] layout: no PE transposes, no
    psum->sbuf attn copy.  E tiles for one jq-strip live in SBUF (strict
    per-strip barrier), accumulators are [128,65] psum tiles finalized by
    one reciprocal + one scalar_tensor_tensor (divide + residual add).
  * LayerNorm: DVE bn_stats per 128-row block (interleaved with finalize),
    rstd via ACT Sqrt + DVE reciprocal.  Chunk 0 (mid-kernel, ACT/DVE
    exp-saturated) does the affine on DVE and gamma/beta on the idle
    GPSIMD; chunk 1 (endgame tail, ACT idle) does stage 1 on ACT
    (Identity with per-partition scale/bias), gamma on DVE and beta
    alternating DVE/GPSIMD.
  * PSUM: 6 banks triple-buffer the score pairs, 1+1 banks double-buffer
    the projection pipeline (alternating two pools), the tail attention
    borrows the idle score pool for 3-deep pipelining.
"""
import numpy as np
import ml_dtypes
from contextlib import ExitStack

_STATE = {}


def _imports():
    global bass, bacc, tile, mybir, bass_utils, F32, BF16, I8, E4, E5, ALU, ACTF, DR
    import concourse.bass as bass
    import concourse.bacc as bacc
    import concourse.tile as tile
    from concourse import mybir
    from concourse import bass_utils
    F32 = mybir.dt.float32
    BF16 = mybir.dt.bfloat16
    I8 = mybir.dt.int8
    E4 = mybir.dt.float8e4
    E5 = mybir.dt.float8e5
    ALU = mybir.AluOpType
    ACTF = mybir.ActivationFunctionType
    DR = mybir.MatmulPerfMode.DoubleRow


N_CORES = 8
CH = 2           # chunks per core
S = 512          # rows per chunk
D = 512          # model dim
EPS = 1e-5

# Schraudolph-e5m2 constants (calibrated offline vs true softmax):
#   i8 = round(score * EXP_A + EXP_B); bits are the e5m2 weight
#   ACT path: exp(score/8 + LN_S) in e5m2 matches the Schraudolph scale.
EXP_A = 4 * np.log2(np.e) / 8        # 0.7213475
EXP_B = 58.0
LN_S = -0.3095

# emit-time engine cost estimates (us) for the greedy ACT/DVE balancer
C_ACT_EXP = 1.098
C_DVE_EXP = 1.262
C_ACT_CONV = 0.672
C_DVE_CONV = 0.730
C_DVE_VCONV = 0.80
C_DVE_FIN = 0.40
C_DVE_LN = 5.2


def _emit(nc, tc, ctx):
    xtb_d = nc.dram_tensor("xtb", [CH, 128, 2048], BF16, kind="ExternalInput").ap()
    xf_d = nc.dram_tensor("xf", [CH, 128, 2048], F32, kind="ExternalInput").ap()
    wb_d = {n: nc.dram_tensor(n, [128, 2048], BF16, kind="ExternalInput").ap()
            for n in ("wbq", "wbk", "wbv")}
    bqt_d = nc.dram_tensor("bqt", [128, 4], F32, kind="ExternalInput").ap()
    bkt_d = nc.dram_tensor("bkt", [128, 4], F32, kind="ExternalInput").ap()
    bvb_d = nc.dram_tensor("bvb", [128, D], F32, kind="ExternalInput").ap()
    gb_d = nc.dram_tensor("gb", [128, D], F32, kind="ExternalInput").ap()
    bb_d = nc.dram_tensor("bb", [128, D], F32, kind="ExternalInput").ap()
    y_d = nc.dram_tensor("y", [CH, S, D], F32, kind="ExternalOutput").ap()

    consts = ctx.enter_context(tc.tile_pool(name="consts", bufs=1))
    chunkp = ctx.enter_context(tc.tile_pool(name="chunk", bufs=1))
    epool = ctx.enter_context(tc.tile_pool(name="epool", bufs=3))
    ypool = ctx.enter_context(tc.tile_pool(name="ypool", bufs=4))
    small = ctx.enter_context(tc.tile_pool(name="small", bufs=4))
    ps_proj = ctx.enter_context(tc.tile_pool(name="ps_proj", bufs=1, space="PSUM"))
    ps_score = ctx.enter_context(tc.tile_pool(name="ps_score", bufs=3, space="PSUM"))
    ps_attn = ctx.enter_context(tc.tile_pool(name="ps_attn", bufs=1, space="PSUM"))

    wb = {n: consts.tile([128, 2048], BF16, tag=n, name=f"w_{n}")
          for n in ("wbq", "wbk", "wbv")}
    bqt = consts.tile([128, 4], F32, tag="bqt")
    bkt = consts.tile([128, 4], F32, tag="bkt")
    bvb = consts.tile([128, D], F32, tag="bvb")
    gb = consts.tile([128, D], F32, tag="gb")
    bb = consts.tile([128, D], F32, tag="bb")
    lns = consts.tile([128, 1], F32, tag="lns")

    def consts_dma():
        nc.sync.dma_start(wb["wbk"][:], wb_d["wbk"][:])
        nc.vector.memset(lns[:], LN_S)
        # warm the ACT function table while DMAs stream (1.3us one-time load)
        warm = consts.tile([128, 1], F32, tag="warm")
        nc.scalar.activation(warm[:], lns[:], ACTF.Exp)

    def consts_dma_late():
        nc.sync.dma_start(bkt[:], bkt_d[:])
        nc.sync.dma_start(bqt[:], bqt_d[:])
        nc.sync.dma_start(wb["wbq"][:], wb_d["wbq"][:])
        nc.sync.dma_start(wb["wbv"][:], wb_d["wbv"][:])
        nc.sync.dma_start(bvb[:], bvb_d[:])
        nc.sync.dma_start(gb[:], gb_d[:])
        nc.sync.dma_start(bb[:], bb_d[:])

    # greedy engine balancer (estimated cumulative us per engine)
    bal = {"act": 0.0, "dve": 0.0}

    def pick_engine():
        return "act" if bal["act"] <= bal["dve"] else "dve"

    st = [{} for _ in range(CH)]

    def loads(c):
        s = st[c]
        s["xtb"] = chunkp.tile([128, 2048], BF16, tag=f"xtb_{c}", name=f"xtb{c}")
        s["xf"] = chunkp.tile([128, 2048], F32, tag=f"xf_{c}", name=f"xf{c}")
        nc.sync.dma_start(s["xtb"][:], xtb_d[c])
        s["qT"] = chunkp.tile([128, 2048], BF16, tag=f"qT_{c}", name=f"qT{c}")
        s["qTs"] = chunkp.tile([128, 2048], BF16, tag=f"qTs_{c}", name=f"qTs{c}")
        s["kT"] = chunkp.tile([128, 2048], BF16, tag=f"kT_{c}", name=f"kT{c}")
        s["vp"] = chunkp.tile([128, 4 * 520], E4, tag=f"vp_{c}", name=f"vp{c}")
        s["h"] = chunkp.tile([128, 2048], F32, tag=f"h_{c}", name=f"h{c}")

    def proj(c):
        s = st[c]
        xt = s["xtb"]
        n = 0
        for which in ("k", "q", "v"):
            for t in range(4):
                # alternate between the two single-buffer psum pools so the
                # PE->convert chain is double-buffered
                pool_, tag_ = ((ps_proj, "proj") if n % 2 == 0 else
                               (ps_attn, "pa"))
                n += 1
                pp = pool_.tile([128, D], F32, tag=tag_,
                                name=f"pp{c}_{which}{t}")
                if which == "v":
                    for mt in range(4):
                        nc.tensor.matmul(
                            pp[:],
                            xt[:, 512 * mt + 128 * t:512 * mt + 128 * t + 128],
                            wb["wbv"][:, 512 * mt:512 * (mt + 1)],
                            start=(mt == 0), stop=(mt == 3))
                    blk = s["vp"][:].rearrange("p (t j c) -> p t j c", j=8, c=65)
                    nc.vector.tensor_tensor(
                        blk[:, t, :, 0:64],
                        pp[:].rearrange("p (j c) -> p j c", c=64),
                        bvb[:].rearrange("p (j c) -> p j c", c=64), op=ALU.add)
                    nc.vector.memset(blk[:, t, :, 64], 1.0)
                else:
                    wname = "wbq" if which == "q" else "wbk"
                    dst = s["qT"] if which == "q" else s["kT"]
                    bias = bqt if which == "q" else bkt
                    for mt in range(4):
                        nc.tensor.matmul(
                            pp[:],
                            wb[wname][:, 512 * mt + 128 * t:512 * mt + 128 * t + 128],
                            xt[:, 512 * mt:512 * (mt + 1)],
                            start=(mt == 0), stop=(mt == 3))
                    eng = pick_engine()
                    if eng == "act":
                        nc.scalar.activation(dst[:, 512 * t:512 * (t + 1)],
                                             pp[:], ACTF.Identity,
                                             bias=bias[:, t:t + 1])
                        bal["act"] += C_ACT_CONV
                    else:
                        nc.vector.tensor_scalar(dst[:, 512 * t:512 * (t + 1)],
                                                pp[:], bias[:, t:t + 1], None,
                                                op0=ALU.add)
                        bal["dve"] += C_DVE_CONV
                    if which == "q":
                        # partition-swapped copy so score matmul rhs can sit at
                        # either PE tile row base (baseline qTs trick)
                        nc.sync.dma_start(
                            s["qTs"][64:128, 512 * t:512 * (t + 1)],
                            dst[0:64, 512 * t:512 * (t + 1)])
                        nc.sync.dma_start(
                            s["qTs"][0:64, 512 * t:512 * (t + 1)],
                            dst[64:128, 512 * t:512 * (t + 1)])

    # net fixed non-exp work difference (DVE minus ACT), spread as a
    # per-exp-tile handicap so the greedy split tilts smoothly
    # ~fixed_dve/(CH*128) from the cost estimates; the exact value was
    # tuned against TimelineSim (the schedule landscape is noisy at ~1us)
    handicap = 0.15

    def emit_exp(dst, ps):
        bal["dve"] += handicap
        eng = pick_engine()
        if eng == "act":
            nc.scalar.activation(dst, ps, ACTF.Exp, scale=0.125, bias=lns[:])
            bal["act"] += C_ACT_EXP
        else:
            nc.vector.tensor_scalar(dst.bitcast(I8), ps, EXP_A, EXP_B,
                                    op0=ALU.mult, op1=ALU.add)
            bal["dve"] += C_DVE_EXP

    def strips(c, jp):
        """Scores + exp for jq pair (2jp, 2jp+1): 16 (r,jku) psum pairs.

        bf16 scores, row-packed: jk-even rows via kT[0:64] at PE tile (0,0),
        jk-odd via kT[64:128] at (64,0); qT/qTs supply the rhs at the
        matching partition base (baseline pattern)."""
        s = st[c]
        qT, qTs, kT = s["qT"], s["qTs"], s["kT"]

        def qrhs(jq, par):
            src = qT if (jq % 2) == par else qTs
            return src[64 * par:64 * par + 64,
                       512 * (jq // 2):512 * (jq // 2) + 512]

        ets = {}
        for r in range(4):
            for jku in range(4):
                koff = 512 * jku + 128 * r
                ps0 = ps_score.tile([128, 1024], F32, tag="sps",
                                    name=f"s0_{c}_{jp}_{r}_{jku}")
                ps1 = ps_score.tile([128, 1024], F32, tag="sps",
                                    name=f"s1_{c}_{jp}_{r}_{jku}")
                for pjq in range(2):
                    jq = 2 * jp + pjq
                    nc.tensor.matmul(ps0[:, 512 * pjq:512 * (pjq + 1)],
                                     kT[0:64, koff:koff + 128], qrhs(jq, 0),
                                     start=True, stop=True,
                                     tile_position=(0, 0))
                for pjq in range(2):
                    jq = 2 * jp + pjq
                    nc.tensor.matmul(ps1[:, 512 * pjq:512 * (pjq + 1)],
                                     kT[64:128, koff:koff + 128], qrhs(jq, 1),
                                     start=True, stop=True,
                                     tile_position=(64, 0))
                et = epool.tile([128, 2048], E5, tag=f"e{4 * r + jku}",
                                name=f"e_{c}_{jp}_{r}_{jku}")
                emit_exp(et[:, 0:1024], ps0[:])
                emit_exp(et[:, 1024:2048], ps1[:])
                ets[(r, jku)] = et
        return ets

    def attn_fin(c, jp, ets, use_sps=False):
        """Flipped attention (E^T stationary) + finalize into h.

        use_sps: draw the accumulators from the (then idle) score psum pool
        for 3-deep pipelining — only safe when no more scores will run."""
        s = st[c]
        vp_v = s["vp"][:].rearrange("p (t j c) -> p t j c", j=8, c=65)
        for sb in range(4):
            for pjq in range(2):
                jq = 2 * jp + pjq
                if use_sps:
                    pa = ps_score.tile([128, 1024], F32, tag="sps",
                                       name=f"pa_{c}_{jp}_{sb}_{pjq}")
                else:
                    pa = ps_attn.tile([128, 512], F32, tag="pa",
                                      name=f"pa_{c}_{jp}_{sb}_{pjq}")
                k = 0
                for r in range(4):
                    for jku in range(4):
                        et = ets[(r, jku)]
                        lhsT = et[:].rearrange("p (i m) -> p i m", i=2)[
                            :, :, 512 * pjq + 128 * sb:512 * pjq + 128 * sb + 128]
                        rhs = vp_v[:, r, 2 * jku:2 * jku + 2, :]
                        nc.tensor.matmul(pa[:, 0:65], lhsT, rhs,
                                         start=(k == 0), stop=(k == 15),
                                         skip_group_check=True, perf_mode=DR)
                        k += 1
                rcp = small.tile([128, 1], F32, tag="rcp",
                                 name=f"rcp_{c}_{jp}_{sb}_{pjq}")
                nc.vector.reciprocal(rcp[:], pa[:, 64:65])
                off = 512 * sb + 64 * jq
                nc.vector.scalar_tensor_tensor(
                    s["h"][:, off:off + 64], pa[:, 0:64], rcp[:],
                    s["xf"][:, off:off + 64], op0=ALU.mult, op1=ALU.add)

    def layer_norm(c):
        """LayerNorm on DVE; Newton rsqrt batched across the 4 s-blocks."""
        s = st[c]
        h = s["h"]
        mvall = small.tile([128, 8], F32, tag="mvall", name=f"mv{c}")
        for b in range(4):
            st6 = small.tile([128, 6], F32, tag="st6", name=f"st6_{c}_{b}")
            nc.vector.bn_stats(st6[:], h[:, 512 * b:512 * (b + 1)])
            nc.vector.bn_aggr(mvall[:, 2 * b:2 * b + 2], st6[:])
        mean4 = mvall[:].rearrange("p (b two) -> p b two", two=2)[:, :, 0]
        var4 = mvall[:].rearrange("p (b two) -> p b two", two=2)[:, :, 1]
        t4 = small.tile([128, 4], F32, tag="t4", name=f"t4_{c}")
        nc.vector.tensor_scalar_add(t4[:], var4, EPS)
        sq4 = small.tile([128, 4], F32, tag="sq4", name=f"sq4_{c}")
        nc.scalar.activation(sq4[:], t4[:], ACTF.Sqrt)
        rstd = small.tile([128, 4], F32, tag="rstd", name=f"rstd{c}")
        nc.vector.reciprocal(rstd[:], sq4[:])
        bco = small.tile([128, 4], F32, tag="bco", name=f"bco{c}")
        nc.vector.tensor_tensor(bco[:], mean4, rstd[:], op=ALU.mult)
        nc.vector.tensor_scalar_mul(bco[:], bco[:], -1.0)
        for b in range(4):
            yt = ypool.tile([128, D], F32, tag="yt", name=f"yt{c}_{b}")
            if c == 0:
                # mid-kernel: ACT/DVE are exp-saturated -> affine on DVE
                # (one ts) and gamma/beta on the otherwise idle GPSIMD
                nc.vector.tensor_scalar(yt[:], h[:, 512 * b:512 * (b + 1)],
                                        rstd[:, b:b + 1], bco[:, b:b + 1],
                                        op0=ALU.mult, op1=ALU.add)
                nc.gpsimd.tensor_tensor(yt[:], yt[:], gb[:], op=ALU.mult)
                nc.gpsimd.tensor_tensor(yt[:], yt[:], bb[:], op=ALU.add)
            else:
                # endgame tail: stage 1 on the idle ACT, gamma on DVE,
                # beta alternating DVE/GPSIMD
                nc.scalar.activation(yt[:], h[:, 512 * b:512 * (b + 1)],
                                     ACTF.Identity, bias=bco[:, b:b + 1],
                                     scale=rstd[:, b:b + 1])
                nc.vector.tensor_tensor(yt[:], yt[:], gb[:], op=ALU.mult)
                eng_bb = nc.gpsimd if b % 2 == 0 else nc.vector
                eng_bb.tensor_tensor(yt[:], yt[:], bb[:], op=ALU.add)
            nc.sync.dma_start(y_d[c, 128 * b:128 * (b + 1), :], yt[:])

    # ---- emission schedule: PE order = proj(0), scores(0,0..3) with
    # attn(jp-1) slotted between strips, proj(1), attn(0,3), scores(1,*) ...
    # The final strips gets an extra DVE handicap so ACT absorbs more of the
    # last exp wave while DVE runs finalize+LN with nothing after it.
    consts_dma()
    loads(0)
    consts_dma_late()
    proj(0)
    loads(1)
    nc.sync.dma_start(st[0]["xf"][:], xf_d[0])
    nc.sync.dma_start(st[1]["xf"][:], xf_d[1])
    pend = None   # (c, jp, ets) awaiting attn+finalize
    for c in range(CH):
        if c == 1:
            proj(1)
            attn_fin(*pend)
            pend = None
            layer_norm(0)
        for jp in range(4):
            ets = strips(c, jp)
            if pend is not None:
                attn_fin(*pend)
            pend = (c, jp, ets)
    attn_fin(*pend, use_sps=True)
    layer_norm(1)


def build():
    if "nc" in _STATE:
        return _STATE["nc"]
    _imports()
    nc = bacc.Bacc("TRN2", target_bir_lowering=False, debug=False,
                   num_devices=N_CORES)
    with tile.TileContext(nc) as tc:
        with ExitStack() as ctx:
            _emit(nc, tc, ctx)
    nc.compile()
    _STATE["nc"] = nc
    return nc


def host_inputs(Wq, bq, Wk, bk, Wv, bv, gamma, beta):
    """Shared per-core constant inputs (everything except x chunks)."""
    bf = ml_dtypes.bfloat16

    def pack_w(W):
        # [p, (mt, m)]: row 128*mt + p of W at free offset 512*mt + m
        W = np.asarray(W, np.float32)
        return np.ascontiguousarray(
            W.reshape(4, 128, 512).transpose(1, 0, 2).reshape(128, 2048)
        ).astype(bf)

    def bias_t(b):
        return np.ascontiguousarray(
            np.asarray(b, np.float32).reshape(4, 128).T)

    return {
        "wbq": pack_w(Wq), "wbk": pack_w(Wk), "wbv": pack_w(Wv),
        "bqt": bias_t(bq), "bkt": bias_t(bk),
        "bvb": np.broadcast_to(np.asarray(bv, np.float32), (128, D)).copy(),
        "gb": np.broadcast_to(np.asarray(gamma, np.float32), (128, D)).copy(),
        "bb": np.broadcast_to(np.asarray(beta, np.float32), (128, D)).copy(),
    }


def kernel(x, Wq, bq, Wk, bk, Wv, bv, gamma, beta):
    _imports()
    nc = build()
    bf = ml_dtypes.bfloat16
    x = np.asarray(x, np.float32)
    B, Sfull, Dm = x.shape
    chunks = x.reshape(B * 8, S, D)  # chunk c = (b = c//8, head = c%8)
    base = host_inputs(Wq=Wq, bq=bq, Wk=Wk, bk=bk, Wv=Wv, bv=bv,
                       gamma=gamma, beta=beta)
    in_maps = []
    for i in range(N_CORES):
        xc = chunks[2 * i:2 * i + 2]                       # [2, 512, 512]
        xtb = np.ascontiguousarray(
            xc.transpose(0, 2, 1).reshape(CH, 4, 128, S).transpose(0, 2, 1, 3)
            .reshape(CH, 128, 2048)).astype(bf)            # x^T, m-tile-major
        xf = np.ascontiguousarray(
            xc.reshape(CH, 4, 128, D).transpose(0, 2, 1, 3)
            .reshape(CH, 128, 2048))                       # residual, s-block-major
        m = dict(base)
        m["xtb"] = xtb
        m["xf"] = xf
        in_maps.append(m)
    res = bass_utils.run_bass_kernel_spmd(nc, in_maps, core_ids=list(range(N_CORES)))
    out_chunks = np.empty((B * 8, S, D), np.float32)
    for i in range(N_CORES):
        out_chunks[2 * i:2 * i + 2] = res.results[i]["y"]
    return out_chunks.reshape(B, Sfull, Dm)


# revision 44
# speedup vs baseline: 1.0229x; 1.0102x over previous
"""Trainium2 Bass kernel for fused MHA block (nn_MultiHeadAttention_7636451852747).

Reference math (B=2, S=4096, D=512, H=8, hd=64):
    q = (x @ Wq + bq).view(B, H, 4096, 64)   # torch-style view, no transpose
    scores = q @ k^T / 8; attn = softmax(scores) @ v -> reshape(B, S, D)
    y = LayerNorm(x + attn) * gamma + beta

Structure: the .view means head h of batch b reads only rows [512h, 512h+512)
of x[b]; the problem splits into 16 independent [512,512] chunks, 2 per core.

Design, driven by the TRN2 timeline cost model:
  * Softmax exp of 16.8M scores/chunk is the bottleneck (only ACT and DVE
    can read PSUM; GPSIMD cannot).  Exp is split across ACT (true exp ->
    fp8e5m2 with bias = ln(scale)) and DVE (Schraudolph bit-trick:
    round(score*EXP_A + EXP_B) as int8 IS the e5m2 weight, scale-matched
    to the ACT path so weights mix within a softmax row).  A greedy
    cost-balancer assigns each exp tile to the less-loaded engine.
  * q/k path runs in bf16 (projection, storage, scores) — fp8 q/k noise
    multiplied into dominant softmax weights and nearly doubled the absmax
    error.  Scores use the 2x row-packed [64,128]x[64,512] layout with a
    partition-swapped qTs copy (tile_position pairing).
  * The attention matmul is fp8 DoubleRow (0.5 cycles/row, 2 k-tiles per
    pass) with E^T e5m2 as the stationary operand and v e4m3 (plus a
    column of ones for the softmax denominator) as the moving operand, so
    the output lands in natural [s, dv] layout: no PE transposes, no
    psum->sbuf attn copy.  E tiles for one jq-strip live in SBUF (strict
    per-strip barrier), accumulators are [128,65] psum tiles finalized by
    one reciprocal + one scalar_tensor_tensor (divide + residual add).
  * LayerNorm: DVE bn_stats per 128-row block (interleaved with finalize),
    rstd via ACT Sqrt + DVE reciprocal.  Chunk 0 (mid-kernel, ACT/DVE
    exp-saturated) does the affine on DVE and gamma/beta on the idle
    GPSIMD; chunk 1 (endgame tail, ACT idle) does stage 1 on ACT
    (Identity with per-partition scale/bias), gamma on DVE and beta
    alternating DVE/GPSIMD.
  * PSUM: 6 banks triple-buffer the score pairs, 1+1 banks double-buffer
    the projection pipeline (alternating two pools), the tail attention
    borrows the idle score pool for 3-deep pipelining.
"""
import numpy as np
import ml_dtypes
from contextlib import ExitStack

_STATE = {}


def _imports():
    global bass, bacc, tile, mybir, bass_utils, F32, BF16, I8, E4, E5, ALU, ACTF, DR
    import concourse.bass as bass
    import concourse.bacc as bacc
    import concourse.tile as tile
    from concourse import mybir
    from concourse import bass_utils
    F32 = mybir.dt.float32
    BF16 = mybir.dt.bfloat16
    I8 = mybir.dt.int8
    E4 = mybir.dt.float8e4
    E5 = mybir.dt.float8e5
    ALU = mybir.AluOpType
    ACTF = mybir.ActivationFunctionType
    DR = mybir.MatmulPerfMode.DoubleRow


N_CORES = 8
CH = 2           # chunks per core
S = 512          # rows per chunk
D = 512          # model dim
EPS = 1e-5

# Schraudolph-e5m2 constants (calibrated offline vs true softmax):
#   i8 = round(score * EXP_A + EXP_B); bits are the e5m2 weight
#   ACT path: exp(score/8 + LN_S) in e5m2 matches the Schraudolph scale.
EXP_A = 4 * np.log2(np.e) / 8        # 0.7213475
EXP_B = 58.0
LN_S = -0.3095

# emit-time engine cost estimates (us) for the greedy ACT/DVE balancer
C_ACT_EXP = 1.098
C_DVE_EXP = 1.262
C_ACT_CONV = 0.672
C_DVE_CONV = 0.730
C_DVE_VCONV = 0.80
C_DVE_FIN = 0.40
C_DVE_LN = 5.2


def _emit(nc, tc, ctx):
    xtb_d = nc.dram_tensor("xtb", [CH, 128, 2048], BF16, kind="ExternalInput").ap()
    xf_d = nc.dram_tensor("xf", [CH, 128, 2048], F32, kind="ExternalInput").ap()
    wb_d = {n: nc.dram_tensor(n, [128, 2048], BF16, kind="ExternalInput").ap()
            for n in ("wbq", "wbk", "wbv")}
    bqt_d = nc.dram_tensor("bqt", [128, 4], F32, kind="ExternalInput").ap()
    bkt_d = nc.dram_tensor("bkt", [128, 4], F32, kind="ExternalInput").ap()
    bvb_d = nc.dram_tensor("bvb", [128, D], F32, kind="ExternalInput").ap()
    gb_d = nc.dram_tensor("gb", [128, D], F32, kind="ExternalInput").ap()
    bb_d = nc.dram_tensor("bb", [128, D], F32, kind="ExternalInput").ap()
    y_d = nc.dram_tensor("y", [CH, S, D], F32, kind="ExternalOutput").ap()

    consts = ctx.enter_context(tc.tile_pool(name="consts", bufs=1))
    chunkp = ctx.enter_context(tc.tile_pool(name="chunk", bufs=1))
    epool = ctx.enter_context(tc.tile_pool(name="epool", bufs=3))
    ypool = ctx.enter_context(tc.tile_pool(name="ypool", bufs=4))
    small = ctx.enter_context(tc.tile_pool(name="small", bufs=4))
    ps_proj = ctx.enter_context(tc.tile_pool(name="ps_proj", bufs=1, space="PSUM"))
    ps_score = ctx.enter_context(tc.tile_pool(name="ps_score", bufs=3, space="PSUM"))
    ps_attn = ctx.enter_context(tc.tile_pool(name="ps_attn", bufs=1, space="PSUM"))

    wb = {n: consts.tile([128, 2048], BF16, tag=n, name=f"w_{n}")
          for n in ("wbq", "wbk", "wbv")}
    bqt = consts.tile([128, 4], F32, tag="bqt")
    bkt = consts.tile([128, 4], F32, tag="bkt")
    bvb = consts.tile([128, D], F32, tag="bvb")
    gb = consts.tile([128, D], F32, tag="gb")
    bb = consts.tile([128, D], F32, tag="bb")
    lns = consts.tile([128, 1], F32, tag="lns")

    def consts_dma():
        nc.sync.dma_start(wb["wbk"][:, 0:512], wb_d["wbk"][:, 0:512])
        nc.vector.memset(lns[:], LN_S)
        # warm the ACT function table while DMAs stream (1.3us one-time load)
        warm = consts.tile([128, 1], F32, tag="warm")
        nc.scalar.activation(warm[:], lns[:], ACTF.Exp)

    def consts_dma_late():
        nc.sync.dma_start(wb["wbq"][:, 0:512], wb_d["wbq"][:, 0:512])
        nc.sync.dma_start(bkt[:], bkt_d[:])
        nc.sync.dma_start(bqt[:], bqt_d[:])
        nc.sync.dma_start(wb["wbk"][:, 512:1024], wb_d["wbk"][:, 512:1024])
        nc.sync.dma_start(wb["wbq"][:, 512:1024], wb_d["wbq"][:, 512:1024])
        nc.sync.dma_start(wb["wbk"][:, 1024:2048], wb_d["wbk"][:, 1024:2048])
        nc.sync.dma_start(wb["wbq"][:, 1024:2048], wb_d["wbq"][:, 1024:2048])
        nc.sync.dma_start(wb["wbv"][:], wb_d["wbv"][:])
        nc.sync.dma_start(bvb[:], bvb_d[:])
        nc.sync.dma_start(gb[:], gb_d[:])
        nc.sync.dma_start(bb[:], bb_d[:])

    # greedy engine balancer (estimated cumulative us per engine)
    bal = {"act": 0.0, "dve": 0.0}

    def pick_engine():
        return "act" if bal["act"] <= bal["dve"] else "dve"

    st = [{} for _ in range(CH)]

    def loads(c):
        s = st[c]
        s["xtb"] = chunkp.tile([128, 2048], BF16, tag=f"xtb_{c}", name=f"xtb{c}")
        s["xf"] = chunkp.tile([128, 2048], F32, tag=f"xf_{c}", name=f"xf{c}")
        nc.sync.dma_start(s["xtb"][:], xtb_d[c])
        s["qT"] = chunkp.tile([128, 2048], BF16, tag=f"qT_{c}", name=f"qT{c}")
        s["qTs"] = chunkp.tile([128, 2048], BF16, tag=f"qTs_{c}", name=f"qTs{c}")
        s["kT"] = chunkp.tile([128, 2048], BF16, tag=f"kT_{c}", name=f"kT{c}")
        s["vp"] = chunkp.tile([128, 4 * 520], E4, tag=f"vp_{c}", name=f"vp{c}")
        s["h"] = chunkp.tile([128, 2048], F32, tag=f"h_{c}", name=f"h{c}")

    def proj(c):
        s = st[c]
        xt = s["xtb"]
        n = 0
        for which in ("k", "q", "v"):
            for t in range(4):
                # alternate between the two single-buffer psum pools so the
                # PE->convert chain is double-buffered
                pool_, tag_ = ((ps_proj, "proj") if n % 2 == 0 else
                               (ps_attn, "pa"))
                n += 1
                pp = pool_.tile([128, D], F32, tag=tag_,
                                name=f"pp{c}_{which}{t}")
                if which == "v":
                    for mt in range(4):
                        nc.tensor.matmul(
                            pp[:],
                            xt[:, 512 * mt + 128 * t:512 * mt + 128 * t + 128],
                            wb["wbv"][:, 512 * mt:512 * (mt + 1)],
                            start=(mt == 0), stop=(mt == 3))
                    blk = s["vp"][:].rearrange("p (t j c) -> p t j c", j=8, c=65)
                    nc.vector.tensor_tensor(
                        blk[:, t, :, 0:64],
                        pp[:].rearrange("p (j c) -> p j c", c=64),
                        bvb[:].rearrange("p (j c) -> p j c", c=64), op=ALU.add)
                    nc.vector.memset(blk[:, t, :, 64], 1.0)
                else:
                    wname = "wbq" if which == "q" else "wbk"
                    dst = s["qT"] if which == "q" else s["kT"]
                    bias = bqt if which == "q" else bkt
                    # wbq/wbk are packed [p, (t, mt, m')]: one 512-col block
                    # per t, so the head only waits for the block it needs
                    for mt in range(4):
                        nc.tensor.matmul(
                            pp[:],
                            wb[wname][:, 512 * t + 128 * mt:512 * t + 128 * mt + 128],
                            xt[:, 512 * mt:512 * (mt + 1)],
                            start=(mt == 0), stop=(mt == 3))
                    eng = pick_engine()
                    if eng == "act":
                        nc.scalar.activation(dst[:, 512 * t:512 * (t + 1)],
                                             pp[:], ACTF.Identity,
                                             bias=bias[:, t:t + 1])
                        bal["act"] += C_ACT_CONV
                    else:
                        nc.vector.tensor_scalar(dst[:, 512 * t:512 * (t + 1)],
                                                pp[:], bias[:, t:t + 1], None,
                                                op0=ALU.add)
                        bal["dve"] += C_DVE_CONV
                    if which == "q":
                        # partition-swapped copy so score matmul rhs can sit at
                        # either PE tile row base (baseline qTs trick)
                        nc.sync.dma_start(
                            s["qTs"][64:128, 512 * t:512 * (t + 1)],
                            dst[0:64, 512 * t:512 * (t + 1)])
                        nc.sync.dma_start(
                            s["qTs"][0:64, 512 * t:512 * (t + 1)],
                            dst[64:128, 512 * t:512 * (t + 1)])

    # net fixed non-exp work difference (DVE minus ACT), spread as a
    # per-exp-tile handicap so the greedy split tilts smoothly
    # ~fixed_dve/(CH*128) from the cost estimates; the exact value was
    # tuned against TimelineSim (the schedule landscape is noisy at ~1us)
    handicap = 0.15

    def emit_exp(dst, ps):
        bal["dve"] += handicap
        eng = pick_engine()
        if eng == "act":
            nc.scalar.activation(dst, ps, ACTF.Exp, scale=0.125, bias=lns[:])
            bal["act"] += C_ACT_EXP
        else:
            nc.vector.tensor_scalar(dst.bitcast(I8), ps, EXP_A, EXP_B,
                                    op0=ALU.mult, op1=ALU.add)
            bal["dve"] += C_DVE_EXP

    def strips(c, jp):
        """Scores + exp for jq pair (2jp, 2jp+1): 16 (r,jku) psum pairs.

        bf16 scores, row-packed: jk-even rows via kT[0:64] at PE tile (0,0),
        jk-odd via kT[64:128] at (64,0); qT/qTs supply the rhs at the
        matching partition base (baseline pattern)."""
        s = st[c]
        qT, qTs, kT = s["qT"], s["qTs"], s["kT"]

        def qrhs(jq, par):
            src = qT if (jq % 2) == par else qTs
            return src[64 * par:64 * par + 64,
                       512 * (jq // 2):512 * (jq // 2) + 512]

        ets = {}
        for r in range(4):
            for jku in range(4):
                koff = 512 * jku + 128 * r
                ps0 = ps_score.tile([128, 1024], F32, tag="sps",
                                    name=f"s0_{c}_{jp}_{r}_{jku}")
                ps1 = ps_score.tile([128, 1024], F32, tag="sps",
                                    name=f"s1_{c}_{jp}_{r}_{jku}")
                for pjq in range(2):
                    jq = 2 * jp + pjq
                    nc.tensor.matmul(ps0[:, 512 * pjq:512 * (pjq + 1)],
                                     kT[0:64, koff:koff + 128], qrhs(jq, 0),
                                     start=True, stop=True,
                                     tile_position=(0, 0))
                for pjq in range(2):
                    jq = 2 * jp + pjq
                    nc.tensor.matmul(ps1[:, 512 * pjq:512 * (pjq + 1)],
                                     kT[64:128, koff:koff + 128], qrhs(jq, 1),
                                     start=True, stop=True,
                                     tile_position=(64, 0))
                et = epool.tile([128, 2048], E5, tag=f"e{4 * r + jku}",
                                name=f"e_{c}_{jp}_{r}_{jku}")
                emit_exp(et[:, 0:1024], ps0[:])
                emit_exp(et[:, 1024:2048], ps1[:])
                ets[(r, jku)] = et
        return ets

    def attn_fin(c, jp, ets, use_sps=False):
        """Flipped attention (E^T stationary) + finalize into h.

        use_sps: draw the accumulators from the (then idle) score psum pool
        for 3-deep pipelining — only safe when no more scores will run."""
        s = st[c]
        vp_v = s["vp"][:].rearrange("p (t j c) -> p t j c", j=8, c=65)
        for sb in range(4):
            for pjq in range(2):
                jq = 2 * jp + pjq
                if use_sps:
                    pa = ps_score.tile([128, 1024], F32, tag="sps",
                                       name=f"pa_{c}_{jp}_{sb}_{pjq}")
                else:
                    pa = ps_attn.tile([128, 512], F32, tag="pa",
                                      name=f"pa_{c}_{jp}_{sb}_{pjq}")
                k = 0
                for r in range(4):
                    for jku in range(4):
                        et = ets[(r, jku)]
                        lhsT = et[:].rearrange("p (i m) -> p i m", i=2)[
                            :, :, 512 * pjq + 128 * sb:512 * pjq + 128 * sb + 128]
                        rhs = vp_v[:, r, 2 * jku:2 * jku + 2, :]
                        nc.tensor.matmul(pa[:, 0:65], lhsT, rhs,
                                         start=(k == 0), stop=(k == 15),
                                         skip_group_check=True, perf_mode=DR)
                        k += 1
                rcp = small.tile([128, 1], F32, tag="rcp",
                                 name=f"rcp_{c}_{jp}_{sb}_{pjq}")
                nc.vector.reciprocal(rcp[:], pa[:, 64:65])
                off = 512 * sb + 64 * jq
                nc.vector.scalar_tensor_tensor(
                    s["h"][:, off:off + 64], pa[:, 0:64], rcp[:],
                    s["xf"][:, off:off + 64], op0=ALU.mult, op1=ALU.add)

    def layer_norm(c):
        """LayerNorm on DVE; Newton rsqrt batched across the 4 s-blocks."""
        s = st[c]
        h = s["h"]
        mvall = small.tile([128, 8], F32, tag="mvall", name=f"mv{c}")
        for b in range(4):
            st6 = small.tile([128, 6], F32, tag="st6", name=f"st6_{c}_{b}")
            nc.vector.bn_stats(st6[:], h[:, 512 * b:512 * (b + 1)])
            nc.vector.bn_aggr(mvall[:, 2 * b:2 * b + 2], st6[:])
        mean4 = mvall[:].rearrange("p (b two) -> p b two", two=2)[:, :, 0]
        var4 = mvall[:].rearrange("p (b two) -> p b two", two=2)[:, :, 1]
        t4 = small.tile([128, 4], F32, tag="t4", name=f"t4_{c}")
        nc.vector.tensor_scalar_add(t4[:], var4, EPS)
        sq4 = small.tile([128, 4], F32, tag="sq4", name=f"sq4_{c}")
        nc.scalar.activation(sq4[:], t4[:], ACTF.Sqrt)
        rstd = small.tile([128, 4], F32, tag="rstd", name=f"rstd{c}")
        nc.vector.reciprocal(rstd[:], sq4[:])
        bco = small.tile([128, 4], F32, tag="bco", name=f"bco{c}")
        nc.vector.tensor_tensor(bco[:], mean4, rstd[:], op=ALU.mult)
        nc.vector.tensor_scalar_mul(bco[:], bco[:], -1.0)
        for b in range(4):
            yt = ypool.tile([128, D], F32, tag="yt", name=f"yt{c}_{b}")
            if c == 0:
                # mid-kernel: ACT/DVE are exp-saturated -> affine on DVE
                # (one ts) and gamma/beta on the otherwise idle GPSIMD
                nc.vector.tensor_scalar(yt[:], h[:, 512 * b:512 * (b + 1)],
                                        rstd[:, b:b + 1], bco[:, b:b + 1],
                                        op0=ALU.mult, op1=ALU.add)
                nc.gpsimd.tensor_tensor(yt[:], yt[:], gb[:], op=ALU.mult)
                nc.gpsimd.tensor_tensor(yt[:], yt[:], bb[:], op=ALU.add)
            else:
                # endgame tail: stage 1 on the idle ACT, gamma on DVE,
                # beta alternating DVE/GPSIMD
                nc.scalar.activation(yt[:], h[:, 512 * b:512 * (b + 1)],
                                     ACTF.Identity, bias=bco[:, b:b + 1],
                                     scale=rstd[:, b:b + 1])
                nc.vector.tensor_tensor(yt[:], yt[:], gb[:], op=ALU.mult)
                eng_bb = nc.gpsimd if b % 2 == 0 else nc.vector
                eng_bb.tensor_tensor(yt[:], yt[:], bb[:], op=ALU.add)
            nc.sync.dma_start(y_d[c, 128 * b:128 * (b + 1), :], yt[:])

    # ---- emission schedule: PE order = proj(0), scores(0,0..3) with
    # attn(jp-1) slotted between strips, proj(1), attn(0,3), scores(1,*) ...
    # The final strips gets an extra DVE handicap so ACT absorbs more of the
    # last exp wave while DVE runs finalize+LN with nothing after it.
    consts_dma()
    loads(0)
    consts_dma_late()
    proj(0)
    loads(1)
    nc.sync.dma_start(st[0]["xf"][:], xf_d[0])
    nc.sync.dma_start(st[1]["xf"][:], xf_d[1])
    pend = None   # (c, jp, ets) awaiting attn+finalize
    for c in range(CH):
        if c == 1:
            proj(1)
            attn_fin(*pend)
            pend = None
            layer_norm(0)
        for jp in range(4):
            ets = strips(c, jp)
            if pend is not None:
                attn_fin(*pend)
            pend = (c, jp, ets)
    attn_fin(*pend, use_sps=True)
    layer_norm(1)


def build():
    if "nc" in _STATE:
        return _STATE["nc"]
    _imports()
    nc = bacc.Bacc("TRN2", target_bir_lowering=False, debug=False,
                   num_devices=N_CORES)
    with tile.TileContext(nc) as tc:
        with ExitStack() as ctx:
            _emit(nc, tc, ctx)
    nc.compile()
    _STATE["nc"] = nc
    return nc


def host_inputs(Wq, bq, Wk, bk, Wv, bv, gamma, beta):
    """Shared per-core constant inputs (everything except x chunks)."""
    bf = ml_dtypes.bfloat16

    def pack_w(W):
        # [p, (mt, m)]: row 128*mt + p of W at free offset 512*mt + m
        W = np.asarray(W, np.float32)
        return np.ascontiguousarray(
            W.reshape(4, 128, 512).transpose(1, 0, 2).reshape(128, 2048)
        ).astype(bf)

    def pack_w2(W):
        # [p, (t, mt, m')]: row 128*mt + p, col 128*t + m' at free offset
        # 512*t + 128*mt + m' -- groups each projection tile's weights into
        # one contiguous 512-col block so head DMAs can stream per-tile
        W = np.asarray(W, np.float32)
        return np.ascontiguousarray(
            W.reshape(4, 128, 4, 128).transpose(1, 2, 0, 3).reshape(128, 2048)
        ).astype(bf)

    def bias_t(b):
        return np.ascontiguousarray(
            np.asarray(b, np.float32).reshape(4, 128).T)

    return {
        "wbq": pack_w2(Wq), "wbk": pack_w2(Wk), "wbv": pack_w(Wv),
        "bqt": bias_t(bq), "bkt": bias_t(bk),
        "bvb": np.broadcast_to(np.asarray(bv, np.float32), (128, D)).copy(),
        "gb": np.broadcast_to(np.asarray(gamma, np.float32), (128, D)).copy(),
        "bb": np.broadcast_to(np.asarray(beta, np.float32), (128, D)).copy(),
    }


def kernel(x, Wq, bq, Wk, bk, Wv, bv, gamma, beta):
    _imports()
    nc = build()
    bf = ml_dtypes.bfloat16
    x = np.asarray(x, np.float32)
    B, Sfull, Dm = x.shape
    chunks = x.reshape(B * 8, S, D)  # chunk c = (b = c//8, head = c%8)
    base = host_inputs(Wq=Wq, bq=bq, Wk=Wk, bk=bk, Wv=Wv, bv=bv,
                       gamma=gamma, beta=beta)
    in_maps = []
    for i in range(N_CORES):
        xc = chunks[2 * i:2 * i + 2]                       # [2, 512, 512]
        xtb = np.ascontiguousarray(
            xc.transpose(0, 2, 1).reshape(CH, 4, 128, S).transpose(0, 2, 1, 3)
            .reshape(CH, 128, 2048)).astype(bf)            # x^T, m-tile-major
        xf = np.ascontiguousarray(
            xc.reshape(CH, 4, 128, D).transpose(0, 2, 1, 3)
            .reshape(CH, 128, 2048))                       # residual, s-block-major
        m = dict(base)
        m["xtb"] = xtb
        m["xf"] = xf
        in_maps.append(m)
    res = bass_utils.run_bass_kernel_spmd(nc, in_maps, core_ids=list(range(N_CORES)))
    out_chunks = np.empty((B * 8, S, D), np.float32)
    for i in range(N_CORES):
        out_chunks[2 * i:2 * i + 2] = res.results[i]["y"]
    return out_chunks.reshape(B, Sfull, Dm)
